# revision 2
# baseline (speedup 1.0000x reference)
"""Multi-head attention Bass/Tile kernel for Trainium2, 8 cores data-parallel.

Shapes (hardcoded): x [8, 1024, 768], Wqkv [768, 2304], bqkv [2304],
Wproj [768, 768], bproj [768].  B=8 batches -> one batch per NeuronCore.

Per-core dataflow (all matmul operands fp16, PSUM accumulation fp32):
  qT/kT [c, n]  : stationary = W-tiles, moving = xT       (c = head-padded 8*128)
  v     [n, c'] : stationary = xT-tiles, moving = Wv_aug  (c' = 8*(96+1), ones col)
  S^T   [j, i]  : stationary = kT head tile, moving = qT head tile, 2 MMs
                  into one [128,1024] 2-bank psum tile
  expS^T        : ONE ACT exp per (h,j) reading [128,1024] across both banks,
                  fused *E^-0.5 scale, psum->sbuf fp16.  ACT does ONLY exp.
  o_aug^T [d,i] : stationary = v head cols (96 + ones), moving = expS^T,
                  accumulated in a borrowed "mmv" psum slot
                  -> row 96 = softmax denominator (colsum)
  normalize     : DVE recip of colsum row; DMA replicates it across
                  partitions (DRAM bounce, step-0 AP); all-SBUF DVE multiply
  out   [i, e]  : stationary = o_norm^T head tiles, moving = Wproj rows.
                  fp32 out, DMA to DRAM; proj+v biases added on host.

q bias is added during the PSUM->SBUF copy via DVE tensor_scalar_add with a
per-partition bias column (k bias cancels in softmax; v bias folded on host).
All PSUM->SBUF copies run on DVE so ACT is exp-only.
"""

import numpy as np

import concourse.bass as bass
import concourse.bacc as bacc
import concourse.mybir as mybir
import concourse.tile as tile

B, N, E, H = 8, 1024, 768, 8
D = E // H          # 96
DP = 128            # padded head dim (partition tile)
DA = D + 1          # 97: head dim + ones column for colsum
NT = N // 128       # 8 token tiles
ET = E // 128       # 6 embedding k-tiles
SCALE = float(E) ** -0.5

F16 = mybir.dt.float16
F32 = mybir.dt.float32
EXP = mybir.ActivationFunctionType.Exp


def build_program(repeats=1, loop_n=0):
    """loop_n > 0 wraps the body in a hardware For_i loop (timing use)."""
    import contextlib
    nc = bacc.Bacc("TRN2", target_bir_lowering=False)

    xT = nc.dram_tensor("xT", [E, N], F16, kind="ExternalInput")
    wq = nc.dram_tensor("wq", [E, H * DP], F16, kind="ExternalInput")
    wk = nc.dram_tensor("wk", [E, H * DP], F16, kind="ExternalInput")
    wv = nc.dram_tensor("wv", [E, H * DA], F16, kind="ExternalInput")
    wp = nc.dram_tensor("wp", [E, E], F16, kind="ExternalInput")
    bq = nc.dram_tensor("bq", [DP, H], F32, kind="ExternalInput")
    out = nc.dram_tensor("out", [N, E], F32, kind="ExternalOutput")

    with tile.TileContext(nc) as tc:
        with (
            tc.tile_pool(name="persist", bufs=1) as persist,
            tc.tile_pool(name="exps", bufs=2) as exps,
            tc.tile_pool(name="osb", bufs=2) as osb,
            tc.tile_pool(name="outsb", bufs=2) as outp,
            tc.tile_pool(name="mmps", bufs=2, space="PSUM") as mmps,
            tc.tile_pool(name="stps", bufs=2, space="PSUM") as stps,
            tc.tile_pool(name="dramp", bufs=2, space="DRAM") as dramp,
        ):
            loop_cm = (tc.For_i(0, loop_n, 1,
                                hint_engines=tuple(mybir.ALL_ENGINES))
                       if loop_n > 0 else contextlib.nullcontext())
            with loop_cm:
             for _rep in range(repeats):
                # ---------------- load inputs ----------------
                # DMA order = first-use order: x+wv (v phase) first, x in column
                # chunks so the first v matmuls start as soon as cols land
                x_sb, wq_sb, wk_sb, wv_sb = [], [], [], []
                for k in range(ET):
                    xk = persist.tile([128, N], F16, tag=f"x{k}", name=f"x{k}")
                    nc.sync.dma_start(out=xk, in_=xT[k * 128:(k + 1) * 128, :])
                    x_sb.append(xk)
                    vk = persist.tile([128, H * DA], F16, tag=f"wv{k}", name=f"wv{k}")
                    nc.sync.dma_start(out=vk, in_=wv[k * 128:(k + 1) * 128, :])
                    wv_sb.append(vk)
                for k in range(ET):
                    qk = persist.tile([128, H * DP], F16, tag=f"wq{k}", name=f"wq{k}")
                    nc.sync.dma_start(out=qk, in_=wq[k * 128:(k + 1) * 128, :])
                    wq_sb.append(qk)
                    kk = persist.tile([128, H * DP], F16, tag=f"wk{k}", name=f"wk{k}")
                    nc.sync.dma_start(out=kk, in_=wk[k * 128:(k + 1) * 128, :])
                    wk_sb.append(kk)
                bq_sb = persist.tile([DP, H], F32, tag="bq", name="bq_sb")
                nc.sync.dma_start(out=bq_sb, in_=bq[:, :])
                wp_sb = []
                for h in range(H):
                    ph = persist.tile([D, E], F16, tag=f"wp{h}", name=f"wp{h}")
                    nc.sync.dma_start(out=ph, in_=wp[h * D:(h + 1) * D, :])
                    wp_sb.append(ph)

                # ---------------- QKV projections ----------------
                qT = [persist.tile([128, N], F16, tag=f"qT{c}", name=f"qT{c}")
                      for c in range(H)]
                kT = [persist.tile([128, N], F16, tag=f"kT{c}", name=f"kT{c}")
                      for c in range(H)]
                v_sb = [persist.tile([128, H * DA], F16, tag=f"v{n}", name=f"v{n}")
                        for n in range(NT)]

                # v first (needed by every head's AV): stationary = xT n-tile
                for n in range(NT):
                    ns = slice(n * 128, (n + 1) * 128)
                    for off, w in ((0, 512), (512, H * DA - 512)):
                        # own tag: the first v matmul must not inherit a psum-slot
                        # WAR wait on top of its DMA wait (MM allows 1 sync wait)
                        ps = mmps.tile([128, w], F32, tag="mmv", name="ps_v")
                        for k in range(ET):
                            nc.tensor.matmul(
                                ps, x_sb[k][:, ns], wv_sb[k][:, off:off + w],
                                start=(k == 0), stop=(k == ET - 1))
                        nc.vector.tensor_copy(v_sb[n][:, off:off + w], ps)
                    # ones column per head (colsum trick); softmax makes the
                    # k-bias terms cancel and the v-bias is folded on host
                    nc.vector.memset(
                        v_sb[n].rearrange("p (h a) -> p h a", h=H)[:, :, D], 1.0)

                # emit_qk_group(h, idx): one psum accumulation group (idx
                # 0/1 = q chunks, 2/3 = k chunks).  q bias is added by the
                # DVE copy (per-partition scalar add).
                def emit_qk_group(h, idx):
                    w_sb, dst = ((wq_sb, qT[h]) if idx < 2 else (wk_sb, kT[h]))
                    cs = slice(h * 128, (h + 1) * 128)
                    off = (idx % 2) * 512
                    ps = mmps.tile([128, 512], F32, tag="mm", name="ps_qkv")
                    for k in range(ET):
                        nc.tensor.matmul(
                            ps, w_sb[k][:, cs], x_sb[k][:, off:off + 512],
                            start=(k == 0), stop=(k == ET - 1))
                    if idx < 2:
                        nc.vector.tensor_scalar_add(
                            dst[:, off:off + 512], ps, bq_sb[:, h:h + 1])
                    else:
                        nc.vector.tensor_copy(dst[:, off:off + 512], ps)

                o_norm = [persist.tile([D, N], F16, tag=f"on{h}", name=f"on{h}")
                          for h in range(H)]

                def emit_av(h, ex, off):
                    # one AV chunk of head h into a borrowed "mmv" psum slot
                    # (v-phase and proj-phase are temporally separated)
                    hs = slice(h * DA, (h + 1) * DA)
                    av = mmps.tile([128, 512], F32, tag="mmv", name="av_ps")
                    for j in range(NT):
                        nc.tensor.matmul(
                            av[0:DA, :], v_sb[j][:, hs], ex[j][:, off:off + 512],
                            start=(j == 0), stop=(j == NT - 1))
                    nc.vector.tensor_copy(o_sb[h][:, off:off + 512], av[0:DA, :])

                def emit_norm(h, off):
                    if off == 0:
                        rcp[h] = osb.tile([1, N], F16, tag="rcp",
                                          name=f"rcp{h}")
                        with nc.allow_low_precision(reason="denom ~1e3"):
                            nc.vector.reciprocal(rcp[h], o_sb[h][D:DA, :])
                        # replicate the reciprocal row across partitions on
                        # the (idle) DMA engines instead of a PE broadcast
                        # matmul; the mul is then all-SBUF fp16 (DVE 2x mode)
                        rbc[h] = osb.tile([D, N], F16, tag="rbc",
                                          name=f"rbc{h}")
                        # SBUF APs forbid step-0 partitions; bounce the row
                        # through DRAM, whose APs allow broadcast reads
                        dr = dramp.tile([1, N], F16, tag="drcp",
                                        name=f"drcp{h}")
                        nc.sync.dma_start(out=dr[0:1, :], in_=rcp[h][0:1, :])
                        bcast = bass.AP(
                            tensor=dr.tensor, offset=dr.offset,
                            ap=[[0, D]] + [list(d) for d in dr[0:1, :].ap[1:]])
                        nc.sync.dma_start(out=rbc[h], in_=bcast)
                    nc.vector.tensor_mul(
                        o_norm[h][:, off:off + 512],
                        o_sb[h][0:D, off:off + 512],
                        rbc[h][:, off:off + 512])

                o_sb, rcp, rbc, ex_prev = {}, {}, {}, None
                for idx in range(4):
                    emit_qk_group(0, idx)
                for h in range(H):
                    # S^T+exp for head h; between j-tiles, emit next head's
                    # q/k groups and the PREVIOUS head's AV/norm — the static
                    # PE stream then always has ready matmuls after an
                    # st-slot wait (on HW exp is slower vs MMs than the
                    # scheduler's cost model assumes)
                    o_sb[h] = osb.tile([DA, N], F16, tag="osb", name=f"osb{h}")
                    ex = []
                    for j in range(NT):
                        exj = exps.tile([128, N], F16, tag=f"ex{j}", name=f"ex{h}_{j}")
                        js = slice(j * 128, (j + 1) * 128)
                        st = stps.tile([128, 1024], F32, tag="st", name="st_ps")
                        for off in (0, 512):
                            nc.tensor.matmul(
                                st[:, off:off + 512], kT[h][0:D, js],
                                qT[h][0:D, off:off + 512],
                                start=True, stop=True)
                        nc.scalar.activation(exj, st, EXP, scale=SCALE)
                        ex.append(exj)
                        if h + 1 < H and j % 2 == 0:
                            emit_qk_group(h + 1, j // 2)
                        if ex_prev is not None:
                            if j == 1:
                                emit_av(h - 1, ex_prev, 0)
                            elif j == 3:
                                emit_av(h - 1, ex_prev, 512)
                            elif j == 5:
                                emit_norm(h - 1, 0)
                            elif j == 7:
                                emit_norm(h - 1, 512)
                    ex_prev = ex
                # drain the pipeline: last head's AV + norm
                emit_av(H - 1, ex_prev, 0)
                emit_av(H - 1, ex_prev, 512)
                emit_norm(H - 1, 0)
                emit_norm(H - 1, 512)

                # ---------------- output projection ----------------
                for i in range(NT):
                    isl = slice(i * 128, (i + 1) * 128)
                    for ci, (off, w) in enumerate(((0, 512), (512, E - 512))):
                        # alternate psum tags: attention pools are idle by
                        # now, so borrow mmv slots for deeper pipelining
                        tag = "mm" if (2 * i + ci) % 2 == 0 else "mmv"
                        ps = mmps.tile([128, w], F32, tag=tag, name="ps_proj")
                        for h in range(H):
                            nc.tensor.matmul(
                                ps, o_norm[h][:, isl], wp_sb[h][:, off:off + w],
                                start=(h == 0), stop=(h == H - 1))
                        osb_t = outp.tile([128, w], F32, tag="out", name="out_sb")
                        nc.vector.tensor_copy(osb_t, ps)
                        nc.sync.dma_start(out=out[isl, off:off + w], in_=osb_t)

    nc.compile()
    return nc


def prep_weights(Wqkv, bqkv, Wproj, bproj):
    Wr = np.asarray(Wqkv, np.float32).reshape(E, H, D, 3)
    br = np.asarray(bqkv, np.float32).reshape(H, D, 3)
    wq_full = np.zeros((E, H * DP), np.float32)
    wk_full = np.zeros((E, H * DP), np.float32)
    wv_full = np.zeros((E, H * DA), np.float32)
    bq_full = np.zeros((DP, H), np.float32)
    for h in range(H):
        wq_full[:, h * DP:h * DP + D] = Wr[:, h, :, 0]
        wk_full[:, h * DP:h * DP + D] = Wr[:, h, :, 1]
        wv_full[:, h * DA:h * DA + D] = Wr[:, h, :, 2]
        bq_full[0:D, h] = br[h, :, 0]
    # host-side output bias: attn rows sum to 1, so attn@(v+bv) = attn@v + bv
    # and (o + bv_cat) @ Wproj + bproj = o @ Wproj + bp_eff
    bv_cat = br[:, :, 2].reshape(E)
    bp_eff = bv_cat @ np.asarray(Wproj, np.float64) + np.asarray(bproj, np.float64)
    return {
        "wq": wq_full.astype(np.float16),
        "wk": wk_full.astype(np.float16),
        "wv": wv_full.astype(np.float16),
        "wp": np.asarray(Wproj, np.float32).astype(np.float16),
        "bq": bq_full,
    }, bp_eff.astype(np.float32)


def make_in_maps(x, Wqkv, bqkv, Wproj, bproj):
    x = np.asarray(x, np.float32)
    shared, bp_eff = prep_weights(Wqkv, bqkv, Wproj, bproj)
    make_in_maps.bp_eff = bp_eff
    in_maps = []
    for b in range(B):
        m = {"xT": np.ascontiguousarray(x[b].T).astype(np.float16)}
        m.update(shared)
        in_maps.append(m)
    return in_maps


_prog_cache = []


def kernel(x, Wqkv, bqkv, Wproj, bproj, _run_kwargs=None):
    from concourse.bass_utils import run_bass_kernel_spmd

    in_maps = make_in_maps(x, Wqkv, bqkv, Wproj, bproj)
    if not _prog_cache:
        _prog_cache.append(build_program())
    nc = _prog_cache[0]
    res = run_bass_kernel_spmd(nc, in_maps, core_ids=list(range(B)),
                               **(_run_kwargs or {}))
    out = np.stack([r["out"] for r in res.results], axis=0)
    out = out + make_in_maps.bp_eff
    if _run_kwargs:
        kernel.last_result = res
    return out
